# revision 17
# baseline (speedup 1.0000x reference)
"""GCN layer kernel for 8 Trainium2 NeuronCores.

Computes: out = relu(dinv[:,None] * ((adj+I).T @ (dinv[:,None] * (x@W))) + b)
where dinv = rsqrt(colsum(adj) + 1).

Strategy: shard adj by COLUMNS across the 8 cores. Column block c (together
with the full scaled source features z) is exactly what's needed to produce
output rows [c*2048, (c+1)*2048) -- so no device collectives are needed.
Host preprocessing folds the self-loop (+I) and the output-side dinv scaling
into the adjacency block, and casts it to bf16 (exactly halves the HBM
traffic, which is the roofline for this problem: 16384^2 matrix vs 64-wide
features). Each core then runs a single streaming matmul pass:

    out_c.T[64, 2048] = sum_k z_k.T[64,128] @ As_k[128, 2048]   (f32 PSUM)
    out_c.T = relu(out_c.T + b)                                  (one ACT op)

z (16384x64) is the stationary operand (ldweights), the 1 GB adjacency
streams through as the moving operand straight from contiguous DMA tiles.
"""

import sys

import numpy as np

if "/opt/trn_rl_repo" not in sys.path:
    sys.path.insert(0, "/opt/trn_rl_repo")

import ml_dtypes

N = 16384
F = 64
NCORES = 8
NB = N // NCORES  # 2048 columns (= output rows) per core
P = 128
KT = N // P  # 128 k-tiles of 128 source rows each
MM_N = 512  # moving-operand free dim per matmul (one PSUM bank of f32)
DMA_BATCH = 2  # k-tiles per dma_start
APOOL_BUFS = 8  # in-flight A-tile slots (prefetch depth)
ALT_DMA = False  # alternate A-tile DMAs between the SP and ACT HWDGE rings
ZSCALE_P = 9  # z is scaled by 2^ZSCALE_P before fp8 quantization (dr mode)
MODE = "dr2"  # variant kernel() uses

_BASS_CACHE: dict = {}


def _build_bass(reps: int = 1, mode: str = "full"):
    """Build the per-core Bass module. reps>1 repeats the whole compute
    (same inputs/outputs) inside one NEFF -- used only for benchmarking
    device time independent of dispatch overhead. mode: "full" | "dma"
    (loads only, no matmul) | "mm" (matmuls from a single resident tile,
    1/128th of the DMA traffic)."""
    key = (reps, mode, DMA_BATCH, APOOL_BUFS, ALT_DMA)
    if key in _BASS_CACHE:
        return _BASS_CACHE[key]

    import concourse.mybir as mybir
    import concourse.tile as tile
    from concourse import bacc

    if mode in ("dr", "drdma", "drmm"):
        nc = _build_bass_dr(reps, mode)
        _BASS_CACHE[key] = nc
        return nc
    if mode in ("dr2", "dr2dma", "dr2mm"):
        nc = _build_bass_dr2(reps, mode)
        _BASS_CACHE[key] = nc
        return nc

    nc = bacc.Bacc("TRN2", target_bir_lowering=False, debug=False,
                   num_devices=NCORES)

    fp8 = mode in ("fp8", "fp8pair", "dma8", "mm8")
    pair = mode in ("pair", "fp8pair")
    a_dt = mybir.dt.float8e4 if fp8 else mybir.dt.bfloat16
    # pair mode: two col-group-tiled matmuls run concurrently. Each 512-chunk
    # nn gets its own PSUM bank (columns nn*512) with even chunks on
    # partitions 0-63 and odd chunks on 64-127, so no two accumulation
    # groups share a bank.
    b_p = 2 * F if pair else F       # bias/dinv partition count
    a_in = nc.dram_tensor("a", [N, NB], a_dt, kind="ExternalInput")
    z_in = nc.dram_tensor("z", [P, KT * F], mybir.dt.bfloat16,
                          kind="ExternalInput")
    b_in = nc.dram_tensor("bvec", [b_p, 1], mybir.dt.float32,
                          kind="ExternalInput")
    if fp8:
        d_in = nc.dram_tensor("dinv", [b_p, NB], mybir.dt.float32,
                              kind="ExternalInput")
    o_out = nc.dram_tensor("o", [F, NB], mybir.dt.float32,
                           kind="ExternalOutput")

    kb = DMA_BATCH
    # [KT/kb, 128, kb, NB]: group kb consecutive k-tiles into one DMA
    a_tiles = a_in.ap().rearrange("(g t p) i -> g p t i", t=kb, p=P)

    with tile.TileContext(nc) as tc:
        with (
            tc.tile_pool(name="singles", bufs=1) as singles,
            tc.tile_pool(name="apool", bufs=APOOL_BUFS) as apool,
            tc.tile_pool(name="psum", bufs=1, space="PSUM") as psum_pool,
        ):
            z_sb = singles.tile([P, KT * F], mybir.dt.bfloat16)
            nc.sync.dma_start(z_sb[:], z_in.ap())
            b_sb = singles.tile([b_p, 1], mybir.dt.float32)
            nc.sync.dma_start(b_sb[:], b_in.ap())
            d_sb = None
            if fp8:
                d_sb = singles.tile([b_p, NB], mybir.dt.float32, tag="d_sb")
                nc.sync.dma_start(d_sb[:], d_in.ap())

            mm_tile = None
            if mode in ("mm", "mm8"):
                mm_tile = singles.tile([P, kb, NB], a_dt, tag="mm_tile")
                nc.sync.dma_start(mm_tile[:], a_tiles[0])

            for _rep in range(reps):
                ps = psum_pool.tile([b_p, NB], mybir.dt.float32)

                for g in range(KT // kb):
                    if mode in ("mm", "mm8"):
                        at = mm_tile
                    else:
                        at = apool.tile([P, kb, NB], a_dt)
                        eng = nc.scalar if (ALT_DMA and g % 2) else nc.sync
                        eng.dma_start(at[:], a_tiles[g])
                    if mode in ("dma", "dma8"):
                        continue
                    for t in range(kb):
                        kt = g * kb + t
                        zk = z_sb[:, kt * F:(kt + 1) * F]
                        if pair:
                            for nn in range(NB // MM_N):
                                h = nn % 2
                                nc.tensor.matmul(
                                    ps[h * F:(h + 1) * F,
                                       nn * MM_N:(nn + 1) * MM_N],
                                    lhsT=zk,
                                    rhs=at[:, t, nn * MM_N:(nn + 1) * MM_N],
                                    start=(kt == 0),
                                    stop=(kt == KT - 1),
                                    tile_position=(0, h * F),
                                )
                        else:
                            for nn in range(NB // MM_N):
                                nc.tensor.matmul(
                                    ps[:, nn * MM_N:(nn + 1) * MM_N],
                                    lhsT=zk,
                                    rhs=at[:, t, nn * MM_N:(nn + 1) * MM_N],
                                    start=(kt == 0),
                                    stop=(kt == KT - 1),
                                )

                out_sb = singles.tile([b_p, NB], mybir.dt.float32,
                                      tag="out_sb")
                relu = mybir.ActivationFunctionType.Relu
                if mode in ("dma", "dma8"):
                    nc.vector.tensor_copy(out_sb[:F, :F], z_sb[:F, :F])
                    nc.sync.dma_start(o_out.ap(), out_sb[:F, :])
                elif pair:
                    # touch only the written PSUM quadrants
                    for nn in range(NB // MM_N):
                        h = nn % 2
                        sp = slice(h * F, (h + 1) * F)
                        sf = slice(nn * MM_N, (nn + 1) * MM_N)
                        if fp8:
                            nc.vector.tensor_mul(out_sb[sp, sf], ps[sp, sf],
                                                 d_sb[sp, sf])
                            nc.scalar.activation(out_sb[sp, sf],
                                                 out_sb[sp, sf], relu,
                                                 bias=b_sb[sp], scale=1.0)
                        else:
                            nc.scalar.activation(out_sb[sp, sf], ps[sp, sf],
                                                 relu, bias=b_sb[sp],
                                                 scale=1.0)
                        nc.sync.dma_start(o_out.ap()[:, sf], out_sb[sp, sf])
                elif fp8:
                    nc.vector.tensor_mul(out_sb[:], ps[:], d_sb[:])
                    nc.scalar.activation(out_sb[:], out_sb[:], relu,
                                         bias=b_sb[:], scale=1.0)
                    nc.sync.dma_start(o_out.ap(), out_sb[:])
                else:
                    nc.scalar.activation(out_sb[:], ps[:], relu,
                                         bias=b_sb[:], scale=1.0)
                    nc.sync.dma_start(o_out.ap(), out_sb[:])

    nc.compile()
    _BASS_CACHE[reps] = nc
    return nc


def _build_bass_dr(reps: int = 1, mode: str = "dr"):
    """DoubleRow fp8 variant: both operands fp8e4m3, 0.5 cycles/row.

    z is split as z8 + r8 (fp8 value + fp8-quantized residual, both scaled
    by 2^ZSCALE_P); each A tile [128, 2, NB] is consumed by two DoubleRow
    matmuls per 512-chunk (z8 then r8) accumulating into the same PSUM
    region, so the PE does 2 x 256-deep contractions per tile at 0.5
    cycles/row = 54.6us/pass, fully hidden under the ~94us A-stream DMA.
    The self-loop (+I) stays folded into A; dinv and the 2^-ZSCALE_P are
    applied on PSUM readout. mode: "dr" | "drdma" (loads only) | "drmm"
    (matmuls from one resident tile).
    """
    import concourse.mybir as mybir
    import concourse.tile as tile
    from concourse import bacc

    nc = bacc.Bacc("TRN2", target_bir_lowering=False, debug=False,
                   num_devices=NCORES)

    kb = DMA_BATCH
    assert kb % 2 == 0
    a_dt = mybir.dt.float8e4
    dr = mybir.MatmulPerfMode.DoubleRow

    a_in = nc.dram_tensor("a", [N, NB], a_dt, kind="ExternalInput")
    z8_in = nc.dram_tensor("z8", [P, KT * F], a_dt, kind="ExternalInput")
    r8_in = nc.dram_tensor("r8", [P, KT * F], a_dt, kind="ExternalInput")
    b_in = nc.dram_tensor("bvec", [F, 1], mybir.dt.float32,
                          kind="ExternalInput")
    d_in = nc.dram_tensor("dinv", [F, NB], mybir.dt.float32,
                          kind="ExternalInput")
    o_out = nc.dram_tensor("o", [F, NB], mybir.dt.float32,
                           kind="ExternalOutput")

    # [KT/kb, 128, kb, NB]: group kb consecutive k-tiles into one DMA
    a_tiles = a_in.ap().rearrange("(g t p) i -> g p t i", t=kb, p=P)
    NG = KT // kb          # a-tile groups
    NPAIR = kb // 2        # DoubleRow k-pair matmul groups per a-tile
    NCHUNK = NB // MM_N    # 512-wide output chunks

    with tile.TileContext(nc) as tc:
        with (
            tc.tile_pool(name="singles", bufs=1) as singles,
            tc.tile_pool(name="apool", bufs=APOOL_BUFS) as apool,
            tc.tile_pool(name="psum", bufs=2, space="PSUM") as psum_pool,
        ):
            z8_sb = singles.tile([P, KT, F], a_dt, tag="z8_sb")
            nc.sync.dma_start(z8_sb[:], z8_in.ap())
            r8_sb = singles.tile([P, KT, F], a_dt, tag="r8_sb")
            nc.sync.dma_start(r8_sb[:], r8_in.ap())
            b_sb = singles.tile([F, 1], mybir.dt.float32, tag="b_sb")
            nc.sync.dma_start(b_sb[:], b_in.ap())
            d_sb = singles.tile([F, NB], mybir.dt.float32, tag="d_sb")
            nc.sync.dma_start(d_sb[:], d_in.ap())

            mm_tile = None
            if mode == "drmm":
                mm_tile = singles.tile([P, kb, NB], a_dt, tag="mm_tile")
                nc.sync.dma_start(mm_tile[:], a_tiles[0])

            for _rep in range(reps):
                ps = psum_pool.tile([F, NB], mybir.dt.float32)

                for g in range(NG):
                    if mode == "drmm":
                        at = mm_tile
                    else:
                        at = apool.tile([P, kb, NB], a_dt)
                        eng = nc.scalar if (ALT_DMA and g % 2) else nc.sync
                        eng.dma_start(at[:], a_tiles[g])
                    if mode == "drdma":
                        continue
                    for t2 in range(NPAIR):
                        kt0 = (g * kb + 2 * t2) if mode != "drmm" else 2 * t2
                        first = g == 0 and t2 == 0
                        last = g == NG - 1 and t2 == NPAIR - 1
                        for nn in range(NCHUNK):
                            sf = slice(nn * MM_N, (nn + 1) * MM_N)
                            nc.tensor.matmul(
                                ps[:, sf],
                                lhsT=z8_sb[:, kt0:kt0 + 2, :],
                                rhs=at[:, 2 * t2:2 * t2 + 2, sf],
                                start=first,
                                stop=False,
                                perf_mode=dr,
                            )
                            nc.tensor.matmul(
                                ps[:, sf],
                                lhsT=r8_sb[:, kt0:kt0 + 2, :],
                                rhs=at[:, 2 * t2:2 * t2 + 2, sf],
                                start=False,
                                stop=last,
                                perf_mode=dr,
                            )

                out_sb = singles.tile([F, NB], mybir.dt.float32,
                                      tag="out_sb")
                relu = mybir.ActivationFunctionType.Relu
                if mode == "drdma":
                    nc.vector.tensor_copy(out_sb[:, :F], z8_sb[:F, 0, :])
                    nc.sync.dma_start(o_out.ap(), out_sb[:])
                else:
                    nc.vector.tensor_mul(out_sb[:], ps[:], d_sb[:])
                    nc.scalar.activation(out_sb[:], out_sb[:], relu,
                                         bias=b_sb[:], scale=1.0)
                    nc.sync.dma_start(o_out.ap(), out_sb[:])

    nc.compile()
    return nc


def _build_bass_dr2(reps: int = 1, mode: str = "dr2"):
    """DoubleRow with the full 128-wide stationary: [z8 | r8] packed in m.

    The PE runs at 1 output-row/cycle regardless of dtype (measured), so
    the only way to cut cycles is to do more per row. One DoubleRow matmul
    (k=256, m=128, n=512) computes z8^T A into PSUM partitions 0:64 AND
    r8^T A into 64:128 from a single pass of the moving A-tile: 256
    matmuls x 512 cycles = 54.6us/pass, fully hidden under the ~94us
    A-stream DMA. Epilogue: DMA ps[64:128] back to partitions 0:64, DVE
    add + dinv mul, ACT relu+bias.
    """
    import concourse.mybir as mybir
    import concourse.tile as tile
    from concourse import bacc

    nc = bacc.Bacc("TRN2", target_bir_lowering=False, debug=False,
                   num_devices=NCORES)

    kb = DMA_BATCH
    assert kb % 2 == 0
    a_dt = mybir.dt.float8e4
    dr = mybir.MatmulPerfMode.DoubleRow

    a_in = nc.dram_tensor("a", [N, NB], a_dt, kind="ExternalInput")
    zr_in = nc.dram_tensor("zr", [P, KT * 2 * F], a_dt, kind="ExternalInput")
    b_in = nc.dram_tensor("bvec", [F, 1], mybir.dt.float32,
                          kind="ExternalInput")
    d_in = nc.dram_tensor("dinv", [F, NB], mybir.dt.float32,
                          kind="ExternalInput")
    id_in = nc.dram_tensor("ident", [2 * F, F], mybir.dt.float32,
                           kind="ExternalInput")
    o_out = nc.dram_tensor("o", [F, NB], mybir.dt.bfloat16,
                           kind="ExternalOutput")

    a_tiles = a_in.ap().rearrange("(g t p) i -> g p t i", t=kb, p=P)
    NG = KT // kb
    NPAIR = kb // 2
    NCHUNK = NB // MM_N

    with tile.TileContext(nc) as tc:
        with (
            tc.tile_pool(name="singles", bufs=1) as singles,
            tc.tile_pool(name="apool", bufs=APOOL_BUFS) as apool,
            tc.tile_pool(name="psum", bufs=2, space="PSUM") as psum_pool,
        ):
            zr_sb = singles.tile([P, KT, 2 * F], a_dt, tag="zr_sb")
            nc.sync.dma_start(zr_sb[:], zr_in.ap())
            b_sb = singles.tile([F, 1], mybir.dt.float32, tag="b_sb")
            nc.sync.dma_start(b_sb[:], b_in.ap())
            d_sb = singles.tile([F, NB], mybir.dt.float32, tag="d_sb")
            nc.sync.dma_start(d_sb[:], d_in.ap())
            id_sb = singles.tile([2 * F, F], mybir.dt.float32, tag="id_sb")
            nc.sync.dma_start(id_sb[:], id_in.ap())

            mm_tile = None
            if mode == "dr2mm":
                mm_tile = singles.tile([P, kb, NB], a_dt, tag="mm_tile")
                nc.sync.dma_start(mm_tile[:], a_tiles[0])

            for _rep in range(reps):
                ps = psum_pool.tile([2 * F, NB], mybir.dt.float32)

                for g in range(NG):
                    if mode == "dr2mm":
                        at = mm_tile
                    else:
                        at = apool.tile([P, kb, NB], a_dt)
                        eng = nc.scalar if (ALT_DMA and g % 2) else nc.sync
                        eng.dma_start(at[:], a_tiles[g])
                    if mode == "dr2dma":
                        continue
                    for t2 in range(NPAIR):
                        kt0 = (g * kb + 2 * t2) if mode != "dr2mm" else 2 * t2
                        first = g == 0 and t2 == 0
                        last = g == NG - 1 and t2 == NPAIR - 1
                        for nn in range(NCHUNK):
                            sf = slice(nn * MM_N, (nn + 1) * MM_N)
                            nc.tensor.matmul(
                                ps[:, sf],
                                lhsT=zr_sb[:, kt0:kt0 + 2, :],
                                rhs=at[:, 2 * t2:2 * t2 + 2, sf],
                                start=first,
                                stop=last,
                                perf_mode=dr,
                            )

                out_sb = singles.tile([F, NB], mybir.dt.bfloat16,
                                      tag="out_sb")
                relu = mybir.ActivationFunctionType.Relu
                if mode == "dr2dma":
                    nc.vector.tensor_copy(out_sb[:, :F], zr_sb[:F, 0, :F])
                    nc.scalar.dma_start(o_out.ap(), out_sb[:])
                else:
                    # fold ps[64:128] (r8 half) into ps[0:64] via the PE:
                    # ACT copies the hi half to SBUF (partition-aligned),
                    # then 4 exact f32 identity matmuls accumulate it into
                    # the lo-half PSUM region -- no DMA-highway traffic.
                    hi128 = singles.tile([2 * F, NB], mybir.dt.float32,
                                         tag="hi128")
                    nc.scalar.copy(hi128[F:2 * F, :], ps[F:2 * F, :])
                    for nn in range(NCHUNK):
                        sf = slice(nn * MM_N, (nn + 1) * MM_N)
                        nc.tensor.matmul(
                            ps[:F, sf],
                            lhsT=id_sb[F:2 * F, :],
                            rhs=hi128[F:2 * F, sf],
                            start=False,
                            stop=True,
                            skip_group_check=True,
                        )
                    nc.vector.tensor_mul(out_sb[:], ps[:F, :], d_sb[:])
                    nc.scalar.activation(out_sb[:], out_sb[:], relu,
                                         bias=b_sb[:], scale=1.0)
                    nc.scalar.dma_start(o_out.ap(), out_sb[:])

    nc.compile()
    return nc


def _host_prep(x, adj, W, b, mode=None):
    """Host-side sharding/preprocessing -> per-core input maps."""
    if mode is None:
        mode = MODE
    fp8 = mode in ("fp8", "fp8pair", "dma8", "mm8")
    pair = mode in ("pair", "fp8pair")
    x = np.asarray(x, dtype=np.float32)
    adj = np.asarray(adj, dtype=np.float32)
    W = np.asarray(W, dtype=np.float32)
    b = np.asarray(b, dtype=np.float32)

    deg = adj.sum(axis=0) + 1.0
    dinv = np.where(deg > 0, 1.0 / np.sqrt(deg), 0.0).astype(np.float32)

    z = (dinv[:, None] * (x @ W)).astype(np.float32)  # [N, F]

    if mode in ("dr", "drdma", "drmm", "dr2", "dr2dma", "dr2mm"):
        fp8 = ml_dtypes.float8_e4m3
        s = np.float32(2.0 ** ZSCALE_P)
        zs = z * s
        z8 = zs.astype(fp8)
        r8 = (zs - z8.astype(np.float32)).astype(fp8)

        def _kmajor(m):
            return np.ascontiguousarray(
                m.reshape(KT, P, F).transpose(1, 0, 2))  # [P, KT, F]

        z8_km = _kmajor(z8)
        r8_km = _kmajor(r8)
        b_dev = np.ascontiguousarray(b.reshape(F, 1))
        idx = np.arange(NB)
        in_maps = []
        for c in range(NCORES):
            cs = c * NB
            blk = adj[:, cs:cs + NB].copy()
            blk[cs + idx, idx] += 1.0  # self-loop (+I), exact in fp8
            dc = (dinv[cs:cs + NB] / s).astype(np.float32)
            m = {
                "a": blk.astype(fp8),
                "bvec": b_dev,
                "dinv": np.ascontiguousarray(np.broadcast_to(dc, (F, NB))),
            }
            if mode.startswith("dr2"):
                zr = np.concatenate([z8_km, r8_km], axis=2)  # [P, KT, 2F]
                m["zr"] = np.ascontiguousarray(zr.reshape(P, KT * 2 * F))
                ident = np.zeros((2 * F, F), np.float32)
                ident[F + np.arange(F), np.arange(F)] = 1.0
                m["ident"] = ident
            else:
                m["z8"] = np.ascontiguousarray(z8_km.reshape(P, KT * F))
                m["r8"] = np.ascontiguousarray(r8_km.reshape(P, KT * F))
            in_maps.append(m)
        return in_maps
    # k-major layout: z_sb[p, kt*F + f] = z[kt*128 + p, f]
    z_dev = np.ascontiguousarray(
        z.reshape(KT, P, F).transpose(1, 0, 2).reshape(P, KT * F)
    ).astype(ml_dtypes.bfloat16)

    if pair:
        b_dev = np.ascontiguousarray(
            np.concatenate([b, b]).reshape(2 * F, 1))
    else:
        b_dev = np.ascontiguousarray(b.reshape(F, 1))

    def _pair_dinv(dc):
        # [128, NB]: chunk nn lives at [64*(nn%2):64*(nn%2+1), nn*512:...]
        d = np.zeros((2 * F, NB), np.float32)
        for nn in range(NB // MM_N):
            h = nn % 2
            d[h * F:(h + 1) * F, nn * MM_N:(nn + 1) * MM_N] = \
                dc[nn * MM_N:(nn + 1) * MM_N]
        return d

    in_maps = []
    idx = np.arange(NB)
    for c in range(NCORES):
        cs = c * NB
        if fp8:
            # adjacency stays exact {0,1,2} in fp8; dinv applied on device
            blk = adj[:, cs:cs + NB].copy()
            blk[cs + idx, idx] += 1.0  # self-loop (+I)
            dc = dinv[cs:cs + NB]
            m = {
                "a": blk.astype(ml_dtypes.float8_e4m3),
                "z": z_dev,
                "bvec": b_dev,
                "dinv": (_pair_dinv(dc) if pair else np.ascontiguousarray(
                    np.broadcast_to(dc, (F, NB)))),
            }
        else:
            blk = adj[:, cs:cs + NB] * dinv[cs:cs + NB][None, :]
            blk[cs + idx, idx] += dinv[cs + idx]  # fold self-loop (+I)
            m = {
                "a": blk.astype(ml_dtypes.bfloat16),
                "z": z_dev,
                "bvec": b_dev,
            }
        in_maps.append(m)
    return in_maps


def _assemble(results, mode=None):
    """Device outputs -> full [N, F] output."""
    if mode is None:
        mode = MODE
    out = np.empty((N, F), dtype=np.float32)
    for c in range(NCORES):
        out[c * NB:(c + 1) * NB, :] = results[c]["o"].T
    return out


def kernel(x, adj, W, b):
    from concourse import bass_utils

    nc = _build_bass(mode=MODE)
    in_maps = _host_prep(x, adj, W, b, mode=MODE)
    res = bass_utils.run_bass_kernel_spmd(nc, in_maps,
                                          core_ids=list(range(NCORES)))
    return _assemble(res.results, mode=MODE)



# revision 21
# speedup vs baseline: 1.0588x; 1.0588x over previous
"""GCN layer kernel for 8 Trainium2 NeuronCores.

Computes: out = relu(dinv[:,None] * ((adj+I).T @ (dinv[:,None] * (x@W))) + b)
where dinv = rsqrt(colsum(adj) + 1).

Strategy: shard adj by COLUMNS across the 8 cores. Column block c (together
with the full scaled source features z) is exactly what's needed to produce
output rows [c*2048, (c+1)*2048) -- so no device collectives are needed.
Host preprocessing folds the self-loop (+I) and the output-side dinv scaling
into the adjacency block, and casts it to bf16 (exactly halves the HBM
traffic, which is the roofline for this problem: 16384^2 matrix vs 64-wide
features). Each core then runs a single streaming matmul pass:

    out_c.T[64, 2048] = sum_k z_k.T[64,128] @ As_k[128, 2048]   (f32 PSUM)
    out_c.T = relu(out_c.T + b)                                  (one ACT op)

z (16384x64) is the stationary operand (ldweights), the 1 GB adjacency
streams through as the moving operand straight from contiguous DMA tiles.
"""

import sys

import numpy as np

if "/opt/trn_rl_repo" not in sys.path:
    sys.path.insert(0, "/opt/trn_rl_repo")

import ml_dtypes

N = 16384
F = 64
NCORES = 8
NB = N // NCORES  # 2048 columns (= output rows) per core
P = 128
KT = N // P  # 128 k-tiles of 128 source rows each
MM_N = 512  # moving-operand free dim per matmul (one PSUM bank of f32)
DMA_BATCH = 2  # k-tiles per dma_start
APOOL_BUFS = 8  # in-flight A-tile slots (prefetch depth)
ALT_DMA = False  # alternate A-tile DMAs between the SP and ACT HWDGE rings
ZSCALE_P = 9  # z is scaled by 2^ZSCALE_P before fp8 quantization (dr mode)
MODE = "dr2"  # variant kernel() uses

_BASS_CACHE: dict = {}


def _build_bass(reps: int = 1, mode: str = "full"):
    """Build the per-core Bass module. reps>1 repeats the whole compute
    (same inputs/outputs) inside one NEFF -- used only for benchmarking
    device time independent of dispatch overhead. mode: "full" | "dma"
    (loads only, no matmul) | "mm" (matmuls from a single resident tile,
    1/128th of the DMA traffic)."""
    key = (reps, mode, DMA_BATCH, APOOL_BUFS, ALT_DMA)
    if key in _BASS_CACHE:
        return _BASS_CACHE[key]

    import concourse.mybir as mybir
    import concourse.tile as tile
    from concourse import bacc

    if mode in ("dr", "drdma", "drmm"):
        nc = _build_bass_dr(reps, mode)
        _BASS_CACHE[key] = nc
        return nc
    if mode in ("dr2", "dr2dma", "dr2mm"):
        nc = _build_bass_dr2(reps, mode)
        _BASS_CACHE[key] = nc
        return nc

    nc = bacc.Bacc("TRN2", target_bir_lowering=False, debug=False,
                   num_devices=NCORES)

    fp8 = mode in ("fp8", "fp8pair", "dma8", "mm8")
    pair = mode in ("pair", "fp8pair")
    a_dt = mybir.dt.float8e4 if fp8 else mybir.dt.bfloat16
    # pair mode: two col-group-tiled matmuls run concurrently. Each 512-chunk
    # nn gets its own PSUM bank (columns nn*512) with even chunks on
    # partitions 0-63 and odd chunks on 64-127, so no two accumulation
    # groups share a bank.
    b_p = 2 * F if pair else F       # bias/dinv partition count
    a_in = nc.dram_tensor("a", [N, NB], a_dt, kind="ExternalInput")
    z_in = nc.dram_tensor("z", [P, KT * F], mybir.dt.bfloat16,
                          kind="ExternalInput")
    b_in = nc.dram_tensor("bvec", [b_p, 1], mybir.dt.float32,
                          kind="ExternalInput")
    if fp8:
        d_in = nc.dram_tensor("dinv", [b_p, NB], mybir.dt.float32,
                              kind="ExternalInput")
    o_out = nc.dram_tensor("o", [F, NB], mybir.dt.float32,
                           kind="ExternalOutput")

    kb = DMA_BATCH
    # [KT/kb, 128, kb, NB]: group kb consecutive k-tiles into one DMA
    a_tiles = a_in.ap().rearrange("(g t p) i -> g p t i", t=kb, p=P)

    with tile.TileContext(nc) as tc:
        with (
            tc.tile_pool(name="singles", bufs=1) as singles,
            tc.tile_pool(name="apool", bufs=APOOL_BUFS) as apool,
            tc.tile_pool(name="psum", bufs=1, space="PSUM") as psum_pool,
        ):
            z_sb = singles.tile([P, KT * F], mybir.dt.bfloat16)
            nc.sync.dma_start(z_sb[:], z_in.ap())
            b_sb = singles.tile([b_p, 1], mybir.dt.float32)
            nc.sync.dma_start(b_sb[:], b_in.ap())
            d_sb = None
            if fp8:
                d_sb = singles.tile([b_p, NB], mybir.dt.float32, tag="d_sb")
                nc.sync.dma_start(d_sb[:], d_in.ap())

            mm_tile = None
            if mode in ("mm", "mm8"):
                mm_tile = singles.tile([P, kb, NB], a_dt, tag="mm_tile")
                nc.sync.dma_start(mm_tile[:], a_tiles[0])

            for _rep in range(reps):
                ps = psum_pool.tile([b_p, NB], mybir.dt.float32)

                for g in range(KT // kb):
                    if mode in ("mm", "mm8"):
                        at = mm_tile
                    else:
                        at = apool.tile([P, kb, NB], a_dt)
                        eng = nc.scalar if (ALT_DMA and g % 2) else nc.sync
                        eng.dma_start(at[:], a_tiles[g])
                    if mode in ("dma", "dma8"):
                        continue
                    for t in range(kb):
                        kt = g * kb + t
                        zk = z_sb[:, kt * F:(kt + 1) * F]
                        if pair:
                            for nn in range(NB // MM_N):
                                h = nn % 2
                                nc.tensor.matmul(
                                    ps[h * F:(h + 1) * F,
                                       nn * MM_N:(nn + 1) * MM_N],
                                    lhsT=zk,
                                    rhs=at[:, t, nn * MM_N:(nn + 1) * MM_N],
                                    start=(kt == 0),
                                    stop=(kt == KT - 1),
                                    tile_position=(0, h * F),
                                )
                        else:
                            for nn in range(NB // MM_N):
                                nc.tensor.matmul(
                                    ps[:, nn * MM_N:(nn + 1) * MM_N],
                                    lhsT=zk,
                                    rhs=at[:, t, nn * MM_N:(nn + 1) * MM_N],
                                    start=(kt == 0),
                                    stop=(kt == KT - 1),
                                )

                out_sb = singles.tile([b_p, NB], mybir.dt.float32,
                                      tag="out_sb")
                relu = mybir.ActivationFunctionType.Relu
                if mode in ("dma", "dma8"):
                    nc.vector.tensor_copy(out_sb[:F, :F], z_sb[:F, :F])
                    nc.sync.dma_start(o_out.ap(), out_sb[:F, :])
                elif pair:
                    # touch only the written PSUM quadrants
                    for nn in range(NB // MM_N):
                        h = nn % 2
                        sp = slice(h * F, (h + 1) * F)
                        sf = slice(nn * MM_N, (nn + 1) * MM_N)
                        if fp8:
                            nc.vector.tensor_mul(out_sb[sp, sf], ps[sp, sf],
                                                 d_sb[sp, sf])
                            nc.scalar.activation(out_sb[sp, sf],
                                                 out_sb[sp, sf], relu,
                                                 bias=b_sb[sp], scale=1.0)
                        else:
                            nc.scalar.activation(out_sb[sp, sf], ps[sp, sf],
                                                 relu, bias=b_sb[sp],
                                                 scale=1.0)
                        nc.sync.dma_start(o_out.ap()[:, sf], out_sb[sp, sf])
                elif fp8:
                    nc.vector.tensor_mul(out_sb[:], ps[:], d_sb[:])
                    nc.scalar.activation(out_sb[:], out_sb[:], relu,
                                         bias=b_sb[:], scale=1.0)
                    nc.sync.dma_start(o_out.ap(), out_sb[:])
                else:
                    nc.scalar.activation(out_sb[:], ps[:], relu,
                                         bias=b_sb[:], scale=1.0)
                    nc.sync.dma_start(o_out.ap(), out_sb[:])

    nc.compile()
    _BASS_CACHE[reps] = nc
    return nc


def _build_bass_dr(reps: int = 1, mode: str = "dr"):
    """DoubleRow fp8 variant: both operands fp8e4m3, 0.5 cycles/row.

    z is split as z8 + r8 (fp8 value + fp8-quantized residual, both scaled
    by 2^ZSCALE_P); each A tile [128, 2, NB] is consumed by two DoubleRow
    matmuls per 512-chunk (z8 then r8) accumulating into the same PSUM
    region, so the PE does 2 x 256-deep contractions per tile at 0.5
    cycles/row = 54.6us/pass, fully hidden under the ~94us A-stream DMA.
    The self-loop (+I) stays folded into A; dinv and the 2^-ZSCALE_P are
    applied on PSUM readout. mode: "dr" | "drdma" (loads only) | "drmm"
    (matmuls from one resident tile).
    """
    import concourse.mybir as mybir
    import concourse.tile as tile
    from concourse import bacc

    nc = bacc.Bacc("TRN2", target_bir_lowering=False, debug=False,
                   num_devices=NCORES)

    kb = DMA_BATCH
    assert kb % 2 == 0
    a_dt = mybir.dt.float8e4
    dr = mybir.MatmulPerfMode.DoubleRow

    a_in = nc.dram_tensor("a", [N, NB], a_dt, kind="ExternalInput")
    z8_in = nc.dram_tensor("z8", [P, KT * F], a_dt, kind="ExternalInput")
    r8_in = nc.dram_tensor("r8", [P, KT * F], a_dt, kind="ExternalInput")
    b_in = nc.dram_tensor("bvec", [F, 1], mybir.dt.float32,
                          kind="ExternalInput")
    d_in = nc.dram_tensor("dinv", [F, NB], mybir.dt.float32,
                          kind="ExternalInput")
    o_out = nc.dram_tensor("o", [F, NB], mybir.dt.float32,
                           kind="ExternalOutput")

    # [KT/kb, 128, kb, NB]: group kb consecutive k-tiles into one DMA
    a_tiles = a_in.ap().rearrange("(g t p) i -> g p t i", t=kb, p=P)
    NG = KT // kb          # a-tile groups
    NPAIR = kb // 2        # DoubleRow k-pair matmul groups per a-tile
    NCHUNK = NB // MM_N    # 512-wide output chunks

    with tile.TileContext(nc) as tc:
        with (
            tc.tile_pool(name="singles", bufs=1) as singles,
            tc.tile_pool(name="apool", bufs=APOOL_BUFS) as apool,
            tc.tile_pool(name="psum", bufs=2, space="PSUM") as psum_pool,
        ):
            z8_sb = singles.tile([P, KT, F], a_dt, tag="z8_sb")
            nc.sync.dma_start(z8_sb[:], z8_in.ap())
            r8_sb = singles.tile([P, KT, F], a_dt, tag="r8_sb")
            nc.sync.dma_start(r8_sb[:], r8_in.ap())
            b_sb = singles.tile([F, 1], mybir.dt.float32, tag="b_sb")
            nc.sync.dma_start(b_sb[:], b_in.ap())
            d_sb = singles.tile([F, NB], mybir.dt.float32, tag="d_sb")
            nc.sync.dma_start(d_sb[:], d_in.ap())

            mm_tile = None
            if mode == "drmm":
                mm_tile = singles.tile([P, kb, NB], a_dt, tag="mm_tile")
                nc.sync.dma_start(mm_tile[:], a_tiles[0])

            for _rep in range(reps):
                ps = psum_pool.tile([F, NB], mybir.dt.float32)

                for g in range(NG):
                    if mode == "drmm":
                        at = mm_tile
                    else:
                        at = apool.tile([P, kb, NB], a_dt)
                        eng = nc.scalar if (ALT_DMA and g % 2) else nc.sync
                        eng.dma_start(at[:], a_tiles[g])
                    if mode == "drdma":
                        continue
                    for t2 in range(NPAIR):
                        kt0 = (g * kb + 2 * t2) if mode != "drmm" else 2 * t2
                        first = g == 0 and t2 == 0
                        last = g == NG - 1 and t2 == NPAIR - 1
                        for nn in range(NCHUNK):
                            sf = slice(nn * MM_N, (nn + 1) * MM_N)
                            nc.tensor.matmul(
                                ps[:, sf],
                                lhsT=z8_sb[:, kt0:kt0 + 2, :],
                                rhs=at[:, 2 * t2:2 * t2 + 2, sf],
                                start=first,
                                stop=False,
                                perf_mode=dr,
                            )
                            nc.tensor.matmul(
                                ps[:, sf],
                                lhsT=r8_sb[:, kt0:kt0 + 2, :],
                                rhs=at[:, 2 * t2:2 * t2 + 2, sf],
                                start=False,
                                stop=last,
                                perf_mode=dr,
                            )

                out_sb = singles.tile([F, NB], mybir.dt.float32,
                                      tag="out_sb")
                relu = mybir.ActivationFunctionType.Relu
                if mode == "drdma":
                    nc.vector.tensor_copy(out_sb[:, :F], z8_sb[:F, 0, :])
                    nc.sync.dma_start(o_out.ap(), out_sb[:])
                else:
                    nc.vector.tensor_mul(out_sb[:], ps[:], d_sb[:])
                    nc.scalar.activation(out_sb[:], out_sb[:], relu,
                                         bias=b_sb[:], scale=1.0)
                    nc.sync.dma_start(o_out.ap(), out_sb[:])

    nc.compile()
    return nc


def _build_bass_dr2(reps: int = 1, mode: str = "dr2"):
    """DoubleRow with the full 128-wide stationary: [z8 | r8] packed in m.

    The PE runs at 1 output-row/cycle regardless of dtype (measured), so
    the only way to cut cycles is to do more per row. One DoubleRow matmul
    (k=256, m=128, n=512) computes z8^T A into PSUM partitions 0:64 AND
    r8^T A into 64:128 from a single pass of the moving A-tile: 256
    matmuls x 512 cycles = 54.6us/pass, fully hidden under the ~94us
    A-stream DMA. Epilogue: DMA ps[64:128] back to partitions 0:64, DVE
    add + dinv mul, ACT relu+bias.
    """
    import concourse.mybir as mybir
    import concourse.tile as tile
    from concourse import bacc

    nc = bacc.Bacc("TRN2", target_bir_lowering=False, debug=False,
                   num_devices=NCORES)

    kb = DMA_BATCH
    assert kb % 2 == 0
    a_dt = mybir.dt.float8e4
    dr = mybir.MatmulPerfMode.DoubleRow

    a_in = nc.dram_tensor("a", [N, NB], a_dt, kind="ExternalInput")
    zr_in = nc.dram_tensor("zr", [P, KT * 2 * F], a_dt, kind="ExternalInput")
    b_in = nc.dram_tensor("bvec", [F, 1], mybir.dt.float32,
                          kind="ExternalInput")
    d_in = nc.dram_tensor("dinv", [F, NB], mybir.dt.float32,
                          kind="ExternalInput")
    id_in = nc.dram_tensor("ident", [2 * F, F], mybir.dt.bfloat16,
                           kind="ExternalInput")
    o_out = nc.dram_tensor("o", [F, NB], mybir.dt.bfloat16,
                           kind="ExternalOutput")

    a_tiles = a_in.ap().rearrange("(g t p) i -> g p t i", t=kb, p=P)
    NG = KT // kb
    NPAIR = kb // 2
    NCHUNK = NB // MM_N

    with tile.TileContext(nc) as tc:
        with (
            tc.tile_pool(name="singles", bufs=1) as singles,
            tc.tile_pool(name="apool", bufs=APOOL_BUFS) as apool,
            tc.tile_pool(name="psum", bufs=2, space="PSUM") as psum_pool,
        ):
            zr_sb = singles.tile([P, KT, 2 * F], a_dt, tag="zr_sb")
            nc.sync.dma_start(zr_sb[:], zr_in.ap())
            b_sb = singles.tile([F, 1], mybir.dt.float32, tag="b_sb")
            nc.sync.dma_start(b_sb[:], b_in.ap())
            d_sb = singles.tile([F, NB], mybir.dt.float32, tag="d_sb")
            nc.sync.dma_start(d_sb[:], d_in.ap())
            id_sb = singles.tile([2 * F, F], mybir.dt.bfloat16, tag="id_sb")
            nc.sync.dma_start(id_sb[:], id_in.ap())

            mm_tile = None
            if mode == "dr2mm":
                mm_tile = singles.tile([P, kb, NB], a_dt, tag="mm_tile")
                nc.sync.dma_start(mm_tile[:], a_tiles[0])

            for _rep in range(reps):
                ps = psum_pool.tile([2 * F, NB], mybir.dt.float32)

                for g in range(NG):
                    if mode == "dr2mm":
                        at = mm_tile
                    else:
                        at = apool.tile([P, kb, NB], a_dt)
                        eng = nc.scalar if (ALT_DMA and g % 2) else nc.sync
                        eng.dma_start(at[:], a_tiles[g])
                    if mode == "dr2dma":
                        continue
                    for t2 in range(NPAIR):
                        kt0 = (g * kb + 2 * t2) if mode != "dr2mm" else 2 * t2
                        first = g == 0 and t2 == 0
                        last = g == NG - 1 and t2 == NPAIR - 1
                        for nn in range(NCHUNK):
                            sf = slice(nn * MM_N, (nn + 1) * MM_N)
                            nc.tensor.matmul(
                                ps[:, sf],
                                lhsT=zr_sb[:, kt0:kt0 + 2, :],
                                rhs=at[:, 2 * t2:2 * t2 + 2, sf],
                                start=first,
                                stop=last,
                                perf_mode=dr,
                            )

                out_sb = singles.tile([F, NB], mybir.dt.bfloat16,
                                      tag="out_sb")
                relu = mybir.ActivationFunctionType.Relu
                if mode == "dr2dma":
                    nc.vector.tensor_copy(out_sb[:, :F], zr_sb[:F, 0, :F])
                    nc.scalar.dma_start(o_out.ap(), out_sb[:])
                else:
                    # fold ps[64:128] (r8 half) into ps[0:64] via the PE:
                    # ACT copies the hi half to SBUF (partition-aligned),
                    # then 4 exact f32 identity matmuls accumulate it into
                    # the lo-half PSUM region -- no DMA-highway traffic.
                    hi128 = singles.tile([2 * F, NB], mybir.dt.bfloat16,
                                         tag="hi128")
                    nc.scalar.copy(hi128[F:2 * F, :], ps[F:2 * F, :])
                    for nn in range(NCHUNK):
                        sf = slice(nn * MM_N, (nn + 1) * MM_N)
                        nc.tensor.matmul(
                            ps[:F, sf],
                            lhsT=id_sb[F:2 * F, :],
                            rhs=hi128[F:2 * F, sf],
                            start=False,
                            stop=True,
                            skip_group_check=True,
                        )
                    nc.vector.tensor_mul(out_sb[:], ps[:F, :], d_sb[:])
                    nc.scalar.activation(out_sb[:], out_sb[:], relu,
                                         bias=b_sb[:], scale=1.0)
                    nc.scalar.dma_start(o_out.ap(), out_sb[:])

    nc.compile()
    return nc


def _host_prep(x, adj, W, b, mode=None):
    """Host-side sharding/preprocessing -> per-core input maps."""
    if mode is None:
        mode = MODE
    fp8 = mode in ("fp8", "fp8pair", "dma8", "mm8")
    pair = mode in ("pair", "fp8pair")
    x = np.asarray(x, dtype=np.float32)
    adj = np.asarray(adj, dtype=np.float32)
    W = np.asarray(W, dtype=np.float32)
    b = np.asarray(b, dtype=np.float32)

    deg = adj.sum(axis=0) + 1.0
    dinv = np.where(deg > 0, 1.0 / np.sqrt(deg), 0.0).astype(np.float32)

    z = (dinv[:, None] * (x @ W)).astype(np.float32)  # [N, F]

    if mode in ("dr", "drdma", "drmm", "dr2", "dr2dma", "dr2mm"):
        fp8 = ml_dtypes.float8_e4m3
        s = np.float32(2.0 ** ZSCALE_P)
        zs = z * s
        z8 = zs.astype(fp8)
        r8 = (zs - z8.astype(np.float32)).astype(fp8)

        def _kmajor(m):
            return np.ascontiguousarray(
                m.reshape(KT, P, F).transpose(1, 0, 2))  # [P, KT, F]

        z8_km = _kmajor(z8)
        r8_km = _kmajor(r8)
        b_dev = np.ascontiguousarray(b.reshape(F, 1))
        idx = np.arange(NB)
        in_maps = []
        for c in range(NCORES):
            cs = c * NB
            blk = adj[:, cs:cs + NB].copy()
            blk[cs + idx, idx] += 1.0  # self-loop (+I), exact in fp8
            dc = (dinv[cs:cs + NB] / s).astype(np.float32)
            m = {
                "a": blk.astype(fp8),
                "bvec": b_dev,
                "dinv": np.ascontiguousarray(np.broadcast_to(dc, (F, NB))),
            }
            if mode.startswith("dr2"):
                zr = np.concatenate([z8_km, r8_km], axis=2)  # [P, KT, 2F]
                m["zr"] = np.ascontiguousarray(zr.reshape(P, KT * 2 * F))
                ident = np.zeros((2 * F, F), ml_dtypes.bfloat16)
                ident[F + np.arange(F), np.arange(F)] = 1.0
                m["ident"] = ident
            else:
                m["z8"] = np.ascontiguousarray(z8_km.reshape(P, KT * F))
                m["r8"] = np.ascontiguousarray(r8_km.reshape(P, KT * F))
            in_maps.append(m)
        return in_maps
    # k-major layout: z_sb[p, kt*F + f] = z[kt*128 + p, f]
    z_dev = np.ascontiguousarray(
        z.reshape(KT, P, F).transpose(1, 0, 2).reshape(P, KT * F)
    ).astype(ml_dtypes.bfloat16)

    if pair:
        b_dev = np.ascontiguousarray(
            np.concatenate([b, b]).reshape(2 * F, 1))
    else:
        b_dev = np.ascontiguousarray(b.reshape(F, 1))

    def _pair_dinv(dc):
        # [128, NB]: chunk nn lives at [64*(nn%2):64*(nn%2+1), nn*512:...]
        d = np.zeros((2 * F, NB), np.float32)
        for nn in range(NB // MM_N):
            h = nn % 2
            d[h * F:(h + 1) * F, nn * MM_N:(nn + 1) * MM_N] = \
                dc[nn * MM_N:(nn + 1) * MM_N]
        return d

    in_maps = []
    idx = np.arange(NB)
    for c in range(NCORES):
        cs = c * NB
        if fp8:
            # adjacency stays exact {0,1,2} in fp8; dinv applied on device
            blk = adj[:, cs:cs + NB].copy()
            blk[cs + idx, idx] += 1.0  # self-loop (+I)
            dc = dinv[cs:cs + NB]
            m = {
                "a": blk.astype(ml_dtypes.float8_e4m3),
                "z": z_dev,
                "bvec": b_dev,
                "dinv": (_pair_dinv(dc) if pair else np.ascontiguousarray(
                    np.broadcast_to(dc, (F, NB)))),
            }
        else:
            blk = adj[:, cs:cs + NB] * dinv[cs:cs + NB][None, :]
            blk[cs + idx, idx] += dinv[cs + idx]  # fold self-loop (+I)
            m = {
                "a": blk.astype(ml_dtypes.bfloat16),
                "z": z_dev,
                "bvec": b_dev,
            }
        in_maps.append(m)
    return in_maps


def _assemble(results, mode=None):
    """Device outputs -> full [N, F] output."""
    if mode is None:
        mode = MODE
    out = np.empty((N, F), dtype=np.float32)
    for c in range(NCORES):
        out[c * NB:(c + 1) * NB, :] = results[c]["o"].T
    return out


def kernel(x, adj, W, b):
    from concourse import bass_utils

    nc = _build_bass(mode=MODE)
    in_maps = _host_prep(x, adj, W, b, mode=MODE)
    res = bass_utils.run_bass_kernel_spmd(nc, in_maps,
                                          core_ids=list(range(NCORES)))
    return _assemble(res.results, mode=MODE)

